# revision 74
# baseline (speedup 1.0000x reference)
"""MultiHeadAttention Trainium2 kernel (8-core SPMD), fp8 DoubleRow edition.

Reference computes, per batch b:
  q = (xq @ wq + bq) -> [S, H, D];  k, v likewise
  score[h] = q_h @ k_h^T;  masked with -1e9 where mask==0 BEFORE /sqrt(D)
  attn = softmax(score / 8)
  out = (attn @ v) @ wo + bo

Sharding: 8 cores = (batch b in 0..3) x (query half qh in 0..1).
Each core: 1024 queries x all 8 heads x full 2048 keys of its batch.
Outputs concatenate on host (no cross-core reduce needed).

Key speedups over the bf16 baseline (193.5us -> 170.6us TimelineSim):
  - All projections run as fp8e4m3 DoubleRow matmuls (0.5 cyc/col, 256-row
    contraction/instr = 4x bf16 rate). Accuracy is restored by a
    compensated product: x,w are split hi/lo ON THE HOST (x8 = [hi;lo] in
    one DMA image, no device-side split cost); V accumulates
    xh*wh + xl*wh + xh*wl, Q/K drop the w-correction term (their outputs
    are e4m3-quantized anyway), so w8q/w8k ship hi-only (shorter DMA
    prefix to the first exp). w is quantized at x32 scale (w std 0.02
    sits at the e4m3 subnormal floor, which silently kills the
    lo-compensation); the PSUM->SBUF copies descale by 1/32 for free.
  - Scores (Q@K^T, d=64 contraction) run as fp8 DoubleRow with the second
    row-slot zeroed (Pool memsets the slots during the DMA window):
    2x bf16 rate. Q/K are quantized e4m3 by the PSUM->SBUF copies (DVE
    tensor_scalar: x1/32, +bias per-partition ptr, fp8 out).
  - attn@V stays bf16 (fp8 there needs hi/lo of the exp tiles; the extra
    elementwise passes would swamp DVE/ACT, or cost ~2.5% error raw).
  - ACT does ONLY the 128 exp ops (its ~134us busy sets the pace;
    measured 92-100% utilization mid-stream). All projection copies on
    DVE; final out-proj copies on ACT (idle in the tail).
  - attnV is deferred to kc8+ (2 kcds/kc; hp0: kc10+, 3/kc): during
    kc0-7 the attnV-accumulator PSUM ring is free and hosts the
    projection PSUM tiles, so projections do not steal score-conveyor
    slots (each steal gapped the exp pipeline ~1.4us).
  - The DMA order feeds the score conveyor first (w8q,x8q,w8k,x8k cols
    0-511) and the warmup matmuls are interleaved with the prologue
    projections so the PE pstate ramp never resets (an idle PE restarts
    at 0.65/1.2GHz and post-idle matmuls run up to 4x slow).
  - bv is folded into bo on the host (bo_eff = bo + bv@wo; works because
    sum(attn)=1 after normalize).
  - Normalize per (hp, ab) on qt-merged [65, 1024] PSUM tiles (recip ->
    Pool partition_broadcast -> mul); per-qt on the last hp so the
    out-projection finishes overlap the chains.

Numerics: no max-subtraction needed (scores are O(1.6)); masked entries
are exactly zero via the multiply. 16-bit tensors use fp16; fp32 PSUM
accumulation everywhere. DRAM output fp16. Measured rel err 1.18e-2
(gate 2e-2), dominated by the e4m3 quantization of Q/K feeding the
score matmul.
"""

import sys

for _p in ("/opt/trn_rl_repo",):
    if _p not in sys.path:
        sys.path.insert(0, _p)

import numpy as np
import ml_dtypes

import concourse.bass as bass
from concourse import bacc
import concourse.tile as tile
import concourse.mybir as mybir
from concourse.bass_utils import run_bass_kernel_spmd
from concourse import dve_ops as _dvo
from concourse.dve_spec import C0, C1, C2, One, Spec, Src0, Src1, lower
from concourse.dve_uop import DveOpSpec

# cubic exp(s/8) approximation (max rel err 1.2% at +-5.5 sigma, prob-rms
# 0.44%): p(s) = ((a s + b) s + c) s + 1, times the mask, fused in ONE DVE
# pass. Lets the DVE take over some exp tiles from the saturated ACT.
_EXPC = (0.00029028, 0.00849829, 0.12820904)


def _make_exp3_op():
    name = "EXP3_MASK_ANT"
    for o in _dvo.OPS:
        if o.name == name:
            return o
    spec = Spec(
        body=(((Src0 * C0 + C1) * Src0 + C2) * Src0 + One) * Src1,
        reference=lambda in0, in1, s0, s1, imm2: (
            ((in0 * s0 + s1) * in0 + imm2) * in0 + 1.0
        )
        * in1,
    )
    row = max(_dvo._SUB_OPCODE_FOR_NAME.values()) + 1
    assert row < 0x20
    # register before compile (DveOp.compile resolves opcode by name)
    _dvo._SUB_OPCODE_FOR_NAME[name] = row
    shas = {
        ver: DveOpSpec(
            name=name, opcode=row, uops=lower(spec, ver=ver), rd1_en=True
        ).sha(ver)
        for ver in ("v3", "v4")
    }
    op = _dvo.DveOp(name, spec, subdim=False, uops_sha=shas)
    _dvo.OPS.append(op)
    _dvo.CUSTOM_DVE_SPECS[name] = spec
    return op


EXP3 = _make_exp3_op()

B, S, E = 4, 2048, 512
H, D = 8, 64
HD = H * D  # 512
SQ = S // 2  # queries per core
P = 128
F32 = mybir.dt.float32
BF16 = mybir.dt.float16  # 16-bit tensors use fp16 (11-bit mantissa)
F8 = mybir.dt.float8e4  # e4m3
E4M3 = ml_dtypes.float8_e4m3fn
DR = mybir.MatmulPerfMode.DoubleRow
EXP = mybir.ActivationFunctionType.Exp
MUL = mybir.AluOpType.mult
ADD = mybir.AluOpType.add

N_CORES = 8
EC = E // P  # 4 contraction chunks for projections
HC = HD // P  # 4 hd chunks
KC = S // P  # 16 key chunks
QT2 = SQ // 512  # 2 q-tiles of 512

# Load-balancing of the 32 (hp, kcpair) exp+mask pipelines across engines.
# With attnV deferred to kc8+ the mask product has >= 8 kc (~16us) of
# latency slack, so slow engines can take whole pairs:
#  - POLY_PAIRS: both exps AND the mask run as ONE fused cubic DVE op per
#    kc (EXP3_MASK_ANT) - removes ~2.1us/pair from the saturated ACT.
#  - POOL_PAIRS: the pair-batched mask multiply runs on GPSIMD (4.2us/op
#    but fully latency-hidden now).
# The last pair of each hp (gp%8==7) stays on the fast path: its product
# feeds the hp-end attnV drain -> normalize -> out-projection chain.
# Only pairs 0-3 of each hp have real slack (pair p is consumed at kc8+p
# but produced at kc 2p+1: slack (6-p) kc), and Pool sustains ~2 pairs/hp
# before queueing erases the slack.
# Measured: ANY offload of the exp/mask pipelines couples the score-PSUM
# slot rotation to slower queues and loses more than it saves
# (poly-on-DVE: +8us for 3 pairs; mask-on-Pool: +35us for 8 pairs).
POLY_PAIRS = frozenset()
POOL_PAIRS = frozenset()

# w is quantized at x32 scale: w*0.02 std sits at the e4m3 subnormal floor
# (min normal 2^-6), which silently kills the lo-compensation term
# (residual 2.8% instead of 0.1%). The copies descale by 1/32 for free.
W_SCALE = 32.0
W_INV = 1.0 / W_SCALE


def build_nc(reps: int = 1) -> bass.Bass:
    nc = bacc.Bacc()

    # ---- DRAM I/O (per-core shards, prepared on host) ----
    # x8*/w8* are [hi; lo] fp8 stacks: rows 0..E-1 = e4m3(x), rows E.. =
    # e4m3(x - hi).
    x8q_d = nc.dram_tensor("x8q", [2 * E, SQ], F8, kind="ExternalInput")
    x8k_d = nc.dram_tensor("x8k", [2 * E, S], F8, kind="ExternalInput")
    x8v_d = nc.dram_tensor("x8v", [2 * E, S], F8, kind="ExternalInput")
    maskT_d = nc.dram_tensor("maskT", [S, SQ], BF16, kind="ExternalInput")
    w8q_d = nc.dram_tensor("w8q", [E, HD], F8, kind="ExternalInput")
    w8k_d = nc.dram_tensor("w8k", [E, HD], F8, kind="ExternalInput")
    w8v_d = nc.dram_tensor("w8v", [2 * E, HD], F8, kind="ExternalInput")
    wo_d = nc.dram_tensor("wo_bf", [HD, E], BF16, kind="ExternalInput")
    bq_d = nc.dram_tensor("bq_pp", [P, HC], F32, kind="ExternalInput")
    bk_d = nc.dram_tensor("bk_pp", [P, HC], F32, kind="ExternalInput")
    bo_d = nc.dram_tensor("bo_row", [1, E], BF16, kind="ExternalInput")
    out_d = nc.dram_tensor("out", [SQ, E], BF16, kind="ExternalOutput")

    with tile.TileContext(nc) as tc:
      for rep in range(reps):
        with (
            tc.tile_pool(name=f"singles{rep}", bufs=1) as singles,
            tc.tile_pool(name=f"work{rep}", bufs=3) as work,
            tc.tile_pool(name=f"pm{rep}", bufs=13) as pm_pool,
            tc.tile_pool(name=f"inputs{rep}", bufs=2) as inputs,
            # scores/proj/fout share 2-bank slots: 4 banks; ao: 4 banks.
            tc.tile_pool(name=f"psum_sc{rep}", bufs=2, space="PSUM") as psum_sc,
            tc.tile_pool(name=f"psum_ao{rep}", bufs=2, space="PSUM") as psum_ao,
        ):
            bq_sb = singles.tile([P, HC], F32, tag="bq")
            bk_sb = singles.tile([P, HC], F32, tag="bk")
            bo_sb = singles.tile([1, E], BF16, tag="bo")
            ones_sb = singles.tile([1, P], BF16, tag="ones1")
            nc.vector.memset(ones_sb[:], 1.0)
            # PE warm-up bridging the input-DMA window: back-to-back matmuls
            # so the pstate ramp is paid before the first real matmul.
            ones2_sb = singles.tile([1, 512], BF16, tag="ones2")
            nc.vector.memset(ones2_sb[:], 1.0)
            wu_ps = psum_sc.tile([P, SQ], F32, tag="scores", name="warmup")

            def warmup(n):
                # keep PE busy through the input-DMA window so the pstate
                # ramp doesn't reset (an idle PE restarts at 0.65/1.2 GHz
                # and the first real matmuls crawl at ~4x cost).
                for _wu in range(n):
                    nc.tensor.matmul(
                        wu_ps[:, 0:512],
                        lhsT=ones_sb[:],
                        rhs=ones2_sb[:],
                        start=True,
                        stop=True,
                    )

            warmup(10)

            # fp8 Q/K with DoubleRow zero-slot layout: [P, HC, 2, n];
            # slot 0 = values (written by the projection copies), slot 1 = 0.
            Q8_sb = singles.tile([P, HC, 2, SQ], F8, tag="Q8")
            K8_sb = singles.tile([P, HC, 2, S], F8, tag="K8")
            # Pool zeroes the unused slots during the DMA window (idle
            # there); split per hc so the first Q/K copies (~10us) only
            # wait on their own chunk.
            for hc0 in range(HC):
                nc.gpsimd.memset(Q8_sb[:, hc0, 1, :], 0.0)
                nc.gpsimd.memset(K8_sb[:, hc0, 1, :], 0.0)
            V_sb = singles.tile([P, KC, H, 65], BF16, tag="V")
            nc.vector.memset(V_sb[:, :, :, 64:65], 1.0)

            maskT_sb = singles.tile([P, KC, SQ], BF16, tag="maskT")

            def dma_mask(mc):
                nc.sync.dma_start(
                    maskT_sb[:, 4 * mc : 4 * mc + 4, :],
                    maskT_d[512 * mc : 512 * (mc + 1), :].rearrange(
                        "(c p) q -> p c q", p=P
                    ),
                )

            # ---- DMA issue order = service order: feed the score conveyor
            # first (Q then K half 1), then mask chunk 0, then the V path,
            # then the rest. First exp fires at ~13us instead of ~27us.
            w8q_sb = inputs.tile([P, EC, HD], F8, tag="wq", bufs=1)
            nc.sync.dma_start(
                w8q_sb[:], w8q_d[:, :].rearrange("(c p) n -> p c n", p=P)
            )
            x8q_sb = inputs.tile([P, 2 * EC, SQ], F8, tag="xq", bufs=1)
            for qh2 in range(2):
                qs2 = slice(qh2 * 512, (qh2 + 1) * 512)
                nc.sync.dma_start(
                    x8q_sb[:, :, qs2],
                    x8q_d[:, qs2].rearrange("(c p) s -> p c s", p=P),
                )
            nc.sync.dma_start(bq_sb[:], bq_d[:, :])
            nc.sync.dma_start(bk_sb[:], bk_d[:, :])
            w8k_sb = inputs.tile([P, EC, HD], F8, tag="wk", bufs=1)
            nc.sync.dma_start(
                w8k_sb[:], w8k_d[:, :].rearrange("(c p) n -> p c n", p=P)
            )
            x8k_sb = inputs.tile([P, 2 * EC, S], F8, tag="xk", bufs=1)

            def dma_xk_q(q):
                ks = slice(q * 512, (q + 1) * 512)
                nc.sync.dma_start(
                    x8k_sb[:, :, ks], x8k_d[:, ks].rearrange("(c p) s -> p c s", p=P)
                )

            w8v_sb = inputs.tile([P, 2 * EC, HD], F8, tag="wv", bufs=1)
            x8v_sb = inputs.tile([P, 2 * EC, S], F8, tag="xv", bufs=1)

            def dma_xv_q(q):
                vs = slice(q * 512, (q + 1) * 512)
                nc.sync.dma_start(
                    x8v_sb[:, :, vs], x8v_d[:, vs].rearrange("(c p) s -> p c s", p=P)
                )

            dma_xk_q(0)
            dma_xk_q(1)
            dma_mask(0)
            nc.sync.dma_start(
                w8v_sb[:], w8v_d[:, :].rearrange("(c p) n -> p c n", p=P)
            )
            dma_xv_q(0)
            dma_xv_q(1)
            dma_xk_q(2)
            dma_xk_q(3)
            dma_xv_q(2)
            dma_xv_q(3)
            dma_mask(1)
            dma_mask(2)
            dma_mask(3)
            wo_bf = singles.tile([P, HC, E], BF16, tag="wo_bf")
            nc.sync.dma_start(wo_bf[:], wo_d[:, :].rearrange("(c p) n -> p c n", p=P))
            nc.sync.dma_start(bo_sb[:], bo_d[:, :])

            AOT_sb = singles.tile([P, HC, SQ], BF16, tag="AOT")

            # compensated fp8 product terms: (x chunk-base, w chunk-base);
            # hi chunks are 0..3, lo chunks are 4..7 of the [hi;lo] stacks.
            # V needs all 3 terms (its error hits the output directly); Q/K
            # outputs get e4m3-quantized anyway (2.5%), so the w-side
            # correction term buys little there - drop it (PE -5.1us).
            TERMS = ((0, 0), (EC, 0), (0, EC))
            TERMS_QK = ((0, 0), (EC, 0))

            def ptile(pool, name):
                tag = "ao" if pool is psum_ao else "scores"
                return pool.tile([P, 512], F32, tag=tag, name=name)

            def vproj_group(sc, pool):
                # V[sc] = x8v[:, sc*128:...]^T @ w8v  -> [128 keys, 512 hd]
                # mid-priority matmuls (above attnV, below scores): a proj
                # group queued behind an attnV burst extends its PSUM
                # slot-hold and gaps the exp conveyor
                ps = ptile(pool, f"vps{sc}")
                ks = slice(sc * P, (sc + 1) * P)
                with tc.high_priority(offset=50):
                    for h2 in range(2):
                        hs = slice(h2 * 256, (h2 + 1) * 256)
                        n = 0
                        for xa, wa in TERMS:
                            for j in range(2):
                                nc.tensor.matmul(
                                    ps[:, hs],
                                    lhsT=x8v_sb[:, xa + 2 * j : xa + 2 * j + 2, ks],
                                    rhs=w8v_sb[:, wa + 2 * j : wa + 2 * j + 2, hs],
                                    start=(n == 0),
                                    stop=(n == 5),
                                    perf_mode=DR,
                                    tile_position=(0, 0),
                                )
                                n += 1
                with tc.high_priority(offset=250):
                    nc.vector.tensor_scalar(
                        V_sb[:, sc, :, 0:64],
                        ps[:, 0:HD].rearrange("p (h d) -> p h d", d=D),
                        W_INV,
                        None,
                        MUL,
                    )

            def proj_group(hc, dst, w_sb, x_sb, b_sb2, nt, pool):
                # dst[hc] tile [128 hd, 512 q] from fp8 3-term DoubleRow.
                ps = ptile(pool, f"pps{hc}{nt}")
                hs = slice(hc * P, (hc + 1) * P)
                with tc.high_priority(offset=50):
                    for q2 in range(2):
                        qs = slice(nt * 512 + q2 * 256, nt * 512 + (q2 + 1) * 256)
                        pqs = slice(q2 * 256, (q2 + 1) * 256)
                        n = 0
                        nlast = 2 * len(TERMS_QK) - 1
                        for xa, wa in TERMS_QK:
                            for j in range(2):
                                nc.tensor.matmul(
                                    ps[:, pqs],
                                    lhsT=w_sb[:, wa + 2 * j : wa + 2 * j + 2, hs],
                                    rhs=x_sb[:, xa + 2 * j : xa + 2 * j + 2, qs],
                                    start=(n == 0),
                                    stop=(n == nlast),
                                    perf_mode=DR,
                                    tile_position=(0, 0),
                                )
                                n += 1
                with tc.high_priority(offset=250):
                    nc.vector.tensor_scalar(
                        dst[:, hc, 0, nt * 512 : (nt + 1) * 512],
                        ps[:, 0:512],
                        W_INV,
                        b_sb2[:, hc : hc + 1],
                        MUL,
                        ADD,
                    )

            def qproj(hc, nt, pool):
                proj_group(hc, Q8_sb, w8q_sb, x8q_sb, bq_sb, nt, pool)

            def kproj(hc, nt, pool):
                proj_group(hc, K8_sb, w8k_sb, x8k_sb, bk_sb, nt, pool)

            # ---- interleave schedules -------------------------------------
            # attnV for every head-pair is deferred to kc8-15 (2 kcds per
            # kc).  During kc0-7 the attnV-accumulator PSUM ring ("ao" tag)
            # is therefore free, and the projection groups allocate THERE
            # instead of stealing score-conveyor slots (each steal gapped
            # ACT ~1.4us).  Only V9-15 (xv DMA lands ~14-21us) remain as
            # late psum_sc steals in hp0's kc8-14.
            ATTNV_SCHED = {8 + j: [2 * j, 2 * j + 1] for j in range(7)}
            ATTNV_SCHED.update({14: [12, 13, 14], 15: [15]})
            # hp0's ring hosts 4 extra V-projections by deferring its attnV
            # two more kc (3 kcds/kc catch-up); only V11-15 stay as steals
            ATTNV_SCHED0 = {
                10: [0, 1, 2], 11: [3, 4, 5], 12: [6, 7, 8],
                13: [9, 10, 11], 14: [12, 13, 14], 15: [15],
            }

            def attnv_kcds(kc, hp):
                return (ATTNV_SCHED0 if hp == 0 else ATTNV_SCHED).get(kc, [])

            # per-(hp, kc) projection work in the ao ring (kc0-7) or as a
            # psum_sc steal (kc8-14):  int n = vproj(n); ("K", hc, nt) = Q/K
            # group (dst inferred); ("Q", hc, nt).
            PROJ_SCHED = {
                0: {
                    0: [("K", 1, 0)], 1: [("K", 1, 1)],
                    2: [("Q", 1, 0), 0], 3: [("Q", 1, 1), 1],
                    4: [2], 5: [3, ("K", 0, 2)], 6: [4, ("K", 0, 3)],
                    7: [5, 6],
                    8: [7, 8], 9: [9, 10],
                    10: [11], 11: [12], 12: [13], 13: [14], 14: [15],
                },
                1: {
                    0: [("K", 1, 2)], 1: [("K", 1, 3)],
                    2: [("Q", 2, 0)], 3: [("Q", 2, 1)], 4: [("K", 2, 0)],
                    5: [("K", 2, 1)], 6: [("K", 2, 2)], 7: [("K", 2, 3)],
                },
                2: {
                    2: [("Q", 3, 0)], 3: [("Q", 3, 1)], 4: [("K", 3, 0)],
                    5: [("K", 3, 1)], 6: [("K", 3, 2)], 7: [("K", 3, 3)],
                },
                3: {},
            }

            for hp in range(HC):  # head pair = heads 2hp, 2hp+1
                if hp == 0:
                    # prologue: only what the first scores need (Q hc0 all q,
                    # K hc0 keys 0..1023; x8q/x8k-c0/c1 are the first DMAs),
                    # in the ao ring, with warmup matmuls plugging the DMA
                    # gaps so the PE pstate ramp never resets.
                    qproj(0, 0, psum_ao)
                    warmup(3)
                    qproj(0, 1, psum_ao)
                    warmup(5)
                    kproj(0, 0, psum_ao)
                    warmup(3)
                    kproj(0, 1, psum_ao)
                    wu_out = work.tile([1, 64], F32, tag="recip", bufs=2)
                    nc.vector.tensor_copy(wu_out[:], wu_ps[0:1, 0:64])
                aos = None  # allocated lazily at kc8 (ring is proj's till then)
                pmt = {}
                sched = PROJ_SCHED[hp]
                ao_kc = 10 if hp == 0 else 8
                for kc in range(KC):
                    pool = psum_ao if kc < ao_kc else psum_sc
                    for item in sched.get(kc, []):
                        if isinstance(item, int):
                            vproj_group(item, pool)
                        elif item[0] == "Q":
                            qproj(item[1], item[2], pool)
                        else:
                            kproj(item[1], item[2], pool)
                    if kc == ao_kc:
                        # qt-merged accumulators: [65, 1024] per ab
                        aos = [
                            psum_ao.tile([65, SQ], F32, tag="ao",
                                         name=f"ao_{hp}_{ab}")
                            for ab in range(2)
                        ]
                    gp = hp * 8 + kc // 2
                    for ab in range(2):
                        pr0, pr1 = ab * 64, (ab + 1) * 64
                        if (ab, kc // 2) not in pmt:
                            pmt[(ab, kc // 2)] = pm_pool.tile(
                                [P, 2, SQ], BF16, tag="pm", name=f"pm_{ab}_{kc//2}"
                            )
                        pm = pmt[(ab, kc // 2)]
                        sc_ps = psum_sc.tile([P, SQ], F32, tag="scores")
                        # scores are the exp-conveyor feed (213ns/kc) and
                        # must not queue behind attnV bursts (1.7us/kc,
                        # slack-rich by design) on the PE
                        with tc.high_priority(offset=100):
                            for qq in range(4):
                                nc.tensor.matmul(
                                    sc_ps[:, qq * 256 : (qq + 1) * 256],
                                    lhsT=K8_sb[pr0:pr1, hp, :, kc * P : (kc + 1) * P],
                                    rhs=Q8_sb[pr0:pr1, hp, :, qq * 256 : (qq + 1) * 256],
                                    start=True,
                                    stop=True,
                                    perf_mode=DR,
                                    tile_position=(pr0, 0),
                                )
                        if gp in POLY_PAIRS:
                            with tc.high_priority(offset=150):
                                nc.vector._custom_dve(
                                    EXP3,
                                    out=pm[:, kc % 2, :],
                                    in0=sc_ps[:],
                                    in1=maskT_sb[:, kc, :],
                                    s0=_EXPC[0],
                                    s1=_EXPC[1],
                                    imm2=_EXPC[2],
                                )
                        else:
                            # the exps ARE the conveyor: their completion
                            # releases the score-PSUM slots everything else
                            # rotates through
                            with tc.high_priority(offset=300):
                                nc.scalar.activation(
                                    pm[:, kc % 2, :], sc_ps[:], EXP, scale=0.125
                                )
                        if kc // 2 == 7:
                            # last pair: mask per (kc, ab) right after the
                            # exp (ab0's mask overlaps ab1's exp) so kcd14
                            # attnV runs at kc14 and only kcd15 remains in
                            # the post-last-exp drain
                            nc.vector.tensor_tensor(
                                pm[:, kc % 2, :],
                                pm[:, kc % 2, :],
                                maskT_sb[:, kc, :],
                                MUL,
                            )
                    if kc // 2 == 7:
                        pass
                    elif kc % 2 == 1 and gp not in POLY_PAIRS:
                        # pair-batched mask multiply; some pairs on GPSIMD
                        eng = nc.gpsimd if gp in POOL_PAIRS else nc.vector
                        for ab in range(2):
                            pm = pmt[(ab, kc // 2)]
                            eng.tensor_tensor(
                                pm[:, :, :],
                                pm[:, :, :],
                                maskT_sb[:, kc - 1 : kc + 1, :],
                                MUL,
                            )
                    # at kc15 drain qt0 (both ab, both kcd) before qt1 so the
                    # first normalize chains start ~1us earlier
                    if kc == KC - 1:
                        av_order = [
                            (ab, kcd, qt)
                            for qt in range(QT2)
                            for ab in range(2)
                            for kcd in attnv_kcds(kc, hp)
                        ]
                    else:
                        av_order = [
                            (ab, kcd, qt)
                            for ab in range(2)
                            for kcd in attnv_kcds(kc, hp)
                            for qt in range(QT2)
                        ]
                    for ab, kcd, qt in av_order:
                        h = 2 * hp + ab
                        pmd = pmt[(ab, kcd // 2)]
                        nc.tensor.matmul(
                            aos[ab][:, qt * 512 : (qt + 1) * 512],
                            lhsT=V_sb[:, kcd, h, :],
                            rhs=pmd[:, kcd % 2, qt * 512 : (qt + 1) * 512],
                            start=(kcd == 0),
                            stop=(kcd == KC - 1),
                        )

                # out-projection per q-pair: hc0-2 partials start as soon
                # as psum slots free; only hc3 + bias wait on the normalize.
                fouts = {}

                def fout_partial(qp, pool):
                    tag = "ao" if pool is psum_ao else "scores"
                    fps = pool.tile([P, 2, E], F32, tag=tag, name=f"fout{qp}")
                    fouts[qp] = fps
                    for half in range(2):
                        qc = 2 * qp + half
                        for hc in range(HC - 1):
                            nc.tensor.matmul(
                                fps[:, half, :],
                                lhsT=AOT_sb[:, hc, qc * P : (qc + 1) * P],
                                rhs=wo_bf[:, hc, :],
                                start=(hc == 0),
                                stop=False,
                            )

                def fout_finish(qp):
                    fps = fouts[qp]
                    for half in range(2):
                        qc = 2 * qp + half
                        nc.tensor.matmul(
                            fps[:, half, :],
                            lhsT=AOT_sb[:, HC - 1, qc * P : (qc + 1) * P],
                            rhs=wo_bf[:, HC - 1, :],
                            start=False,
                            stop=False,
                        )
                        nc.tensor.matmul(
                            fps[:, half, :],
                            lhsT=ones_sb[:],
                            rhs=bo_sb[:],
                            start=False,
                            stop=True,
                        )
                    # copies on ACT: idle in the tail while DVE runs norms
                    fo = work.tile([P, 2, E], BF16, tag="fout", bufs=2)
                    if qp == 3:
                        # final pair: per-half copy+DMA so the last DMA is
                        # small; split across ACT+DVE so they overlap
                        nc.scalar.copy(fo[:, 0, :], fps[:, 0, :])
                        nc.sync.dma_start(out_d[6 * P : 7 * P, :], fo[:, 0, :])
                        nc.vector.tensor_copy(fo[:, 1, :], fps[:, 1, :])
                        nc.sync.dma_start(out_d[7 * P : 8 * P, :], fo[:, 1, :])
                    else:
                        nc.scalar.copy(fo[:], fps[:])
                        nc.sync.dma_start(
                            out_d[2 * qp * P : 2 * (qp + 1) * P, :].rearrange(
                                "(c p) n -> p c n", p=P
                            ),
                            fo[:],
                        )

                def norm(ab, qs):
                    pr0, pr1 = ab * 64, (ab + 1) * 64
                    with tc.high_priority(offset=200):
                        rc = work.tile([1, SQ], F32, tag="recip", bufs=2)
                        nc.vector.reciprocal(rc[0:1, qs], aos[ab][64:65, qs])
                        rcb = work.tile([64, SQ], F32, tag="rcb", bufs=2)
                        nc.gpsimd.partition_broadcast(rcb[:, qs], rc[0:1, qs])
                        nc.vector.tensor_tensor(
                            AOT_sb[pr0:pr1, hp, qs],
                            aos[ab][0:64, qs],
                            rcb[:, qs],
                            MUL,
                        )

                if hp == HC - 1:
                    # per-qt normalize so fout finishes overlap the chains
                    # (256-wide chunking measured worse: the extra recip/
                    # bcast/mul init+launch overheads beat the earlier
                    # first-finish)
                    fout_partial(0, psum_sc)
                    fout_partial(1, psum_sc)
                    norm(0, slice(0, 512))
                    norm(1, slice(0, 512))
                    fout_finish(0)
                    fout_finish(1)
                    norm(0, slice(512, SQ))
                    norm(1, slice(512, SQ))
                    fout_partial(2, psum_sc)
                    fout_partial(3, psum_sc)
                    fout_finish(2)
                    fout_finish(3)
                else:
                    norm(0, slice(0, SQ))
                    norm(1, slice(0, SQ))

    nc.finalize()
    return nc


_NC_CACHE = {}


def _get_nc(reps: int = 1):
    if reps not in _NC_CACHE:
        _NC_CACHE[reps] = build_nc(reps)
    return _NC_CACHE[reps]


def _hilo(x32: np.ndarray, scale: float = 1.0) -> np.ndarray:
    """[N, M] f32 -> [2N, M] fp8 e4m3 stack [hi; lo] of (x * scale)."""
    xs = x32 * np.float32(scale)
    hi = xs.astype(E4M3)
    lo = (xs - hi.astype(np.float32)).astype(E4M3)
    return np.ascontiguousarray(np.concatenate([hi, lo], axis=0))


def make_in_maps(input_q, input_k, input_v, mask, wq, bq, wk, bk, wv, bv, wo, bo):
    input_q = np.asarray(input_q, np.float32)
    input_k = np.asarray(input_k, np.float32)
    input_v = np.asarray(input_v, np.float32)
    mask = np.asarray(mask)
    f = np.float32
    h = np.float16
    # Q/K use 2-term compensation (x-side only): only the hi half of w is
    # ever read on device, so ship hi-only (halves the w8q/w8k DMA).
    w8q = np.ascontiguousarray(
        (np.asarray(wq, f) * np.float32(W_SCALE)).astype(E4M3)
    )
    w8k = np.ascontiguousarray(
        (np.asarray(wk, f) * np.float32(W_SCALE)).astype(E4M3)
    )
    w8v = _hilo(np.ascontiguousarray(wv, f), W_SCALE)
    wo = np.ascontiguousarray(wo, f)
    bq_pp = np.ascontiguousarray(np.asarray(bq, f).reshape(HC, P).T)
    bk_pp = np.ascontiguousarray(np.asarray(bk, f).reshape(HC, P).T)
    # bv folds into bo: out = (sum_k pm v)/denom + bv (since sum attn = 1),
    # so bo_eff = bo + bv @ wo absorbs it after the out-projection.
    bo_row = (np.asarray(bo, f) + np.asarray(bv, f) @ wo).reshape(1, E).astype(h)
    k8 = [_hilo(np.ascontiguousarray(input_k[b].T)) for b in range(B)]
    v8 = [_hilo(np.ascontiguousarray(input_v[b].T)) for b in range(B)]
    in_maps = []
    for c in range(N_CORES):
        b, qh = c // 2, c % 2
        qs = slice(qh * SQ, (qh + 1) * SQ)
        in_maps.append(
            {
                "x8q": _hilo(np.ascontiguousarray(input_q[b, qs].T)),
                "x8k": k8[b],
                "x8v": v8[b],
                "maskT": np.ascontiguousarray(mask[b, qs].T).astype(np.float16),
                "w8q": w8q,
                "w8k": w8k,
                "w8v": w8v,
                "wo_bf": wo.astype(np.float16),
                "bq_pp": bq_pp,
                "bk_pp": bk_pp,
                "bo_row": bo_row,
            }
        )
    return in_maps


def kernel(input_q, input_k, input_v, mask, wq, bq, wk, bk, wv, bv, wo, bo, **_kw):
    nc = _get_nc()
    in_maps = make_in_maps(
        input_q, input_k, input_v, mask, wq, bq, wk, bk, wv, bv, wo, bo
    )
    res = run_bass_kernel_spmd(nc, in_maps, core_ids=list(range(N_CORES)))
    out = np.empty((B, S, E), np.float32)
    for c in range(N_CORES):
        b, qh = c // 2, c % 2
        out[b, qh * SQ : (qh + 1) * SQ] = res.results[c]["out"].astype(np.float32)
    return out


if __name__ == "__main__":
    print("building...")
    _get_nc()
    print("built ok")
